# revision 11
# baseline (speedup 1.0000x reference)
"""Depthwise 3D conv (3x3x3, SAME, C=64) on 8 Trainium2 NeuronCores.

Strategy
--------
Data-parallel over (batch, h-half): core k handles b = k//2 and output
rows h in [56*(k%2), 56*(k%2)+56), all 16 d frames.

TensorE mapping (8-way tile-packed): the 128x128 PE array runs in 64x32
tile mode - 2 row-groups x 4 column-groups = 8 concurrent tiles. Each
tile computes (4 d, 8 h) output blocks (32 output partitions) from
(6 d, 10 h) input blocks (60 partitions, pi = d_loc*10 + h_loc), with a
60x32 banded stationary B[pi, po] = w[kd, kh, kw, c] applying 9 taps
per streamed column; the 3 kw taps are w-shifts on the moving access
pattern, PSUM-accumulated. Per (channel, kw) the 28 blocks (4 d-blocks
x 7 h-blocks) are spread as 4 blocks/tile on row-group 0 and 3
blocks/tile on row-group 1, so one matmul streams 448 (or 336)
columns. Concurrent tiles give ~1.8x the per-column throughput of the
full-array band scheme, and PSUM quadrants are fully dense (all 128
psum partitions carry real outputs), keeping evacuation cheap.

x is host-gathered (fp16) with full halo duplication in HBM, band
matrices built on host (fp16), device output fp16, host casts back to
fp32.
"""

import json
import sys
import types

if "/opt/trn_rl_repo" not in sys.path:
    sys.path.insert(0, "/opt/trn_rl_repo")

import numpy as np

KD = KH = KW = 3
C = 64
B_FULL, D_FULL, H, W = 4, 16, 112, 112
N_CORES = 8
HH = 56  # output h rows per core
DBO, DBI = 4, 6  # d per block: out frames / in frames
HBO, HBI = 8, 10  # h per block: out rows / in rows
NDB, NHB = 4, 7  # blocks: d 16/4, h 56/8
PB = DBI * HBI  # 60 in-partitions per block
POB = DBO * HBO  # 32 out-partitions per block
NS0, NS1 = 16, 12  # slots on row-group 0 (c_h 0..3) / row-group 1 (c_h 4..6)
CG = 4  # channels per input DMA chunk
OG = 2  # channels per output DMA chunk
F16 = np.float16

_KW_ORDER = [1, 0, 2]  # full-width tap first so PSUM start=True covers all cols


def _legalize_bir(raw: bytes) -> bytes:
    """walrus in this image caps sem waits at 1 per instruction; hoist extra
    waits onto preceding same-engine NoOps (sequencers run them in order)."""
    d = json.loads(raw)
    for fn in d["functions"]:
        for blk in fn["blocks"]:
            out = []
            for inst in blk["instructions"]:
                si = inst.get("sync_info")
                waits = (si or {}).get("on_wait") or []
                if len(waits) > 1:
                    for j, wt in enumerate(waits[:-1]):
                        out.append(
                            {
                                "debug": inst.get("debug", 0),
                                "engine": inst["engine"],
                                "ins": [],
                                "outs": [],
                                "name": f"{inst['name']}-w{j}",
                                "opcode": "NoOp",
                                "sync_info": {"on_wait": [wt], "on_update": []},
                            }
                        )
                    si["on_wait"] = [waits[-1]]
                out.append(inst)
            blk["instructions"] = out
    return json.dumps(d).encode()


def _w_ranges(kw):
    # out[w] += wt[kw] * x[w + kw - 1]
    if kw == 1:
        return 0, W, 0, W
    if kw == 0:
        return 0, W - 1, 1, W
    return 1, W, 0, W - 1


def _build_nc():
    import concourse.bass as bass
    import concourse.mybir as mybir
    import concourse.tile as tile

    nc = bass.Bass()
    # row-group 0: 16 slots (c_h 0..3 x b 0..3); row-group 1: 12 slots
    xa_d = nc.declare_dram_parameter(
        "xa", [PB, C, NS0, W], mybir.dt.float16, isOutput=False
    )
    xb_d = nc.declare_dram_parameter(
        "xb", [PB, C, NS1, W], mybir.dt.float16, isOutput=False
    )
    bm_d = nc.declare_dram_parameter(
        "bm", [128, C, KW, POB], mybir.dt.float16, isOutput=False
    )
    ya_d = nc.declare_dram_parameter(
        "ya", [128, C, 4, W], mybir.dt.float16, isOutput=True
    )
    yb_d = nc.declare_dram_parameter(
        "yb", [128, C, 3, W], mybir.dt.float16, isOutput=True
    )

    with tile.TileContext(nc) as tc:
        with (
            tc.tile_pool(name="xin", bufs=6) as xin_pool,
            tc.tile_pool(name="bmat", bufs=4) as b_pool,
            tc.tile_pool(name="psa", bufs=4, space="PSUM") as psa_pool,
            tc.tile_pool(name="psb", bufs=4, space="PSUM") as psb_pool,
            tc.tile_pool(name="osb", bufs=4) as osb_pool,
        ):
            # warm the PE (HAM un-throttle) during the first DMA wait
            warm = xin_pool.tile([128, 448], mybir.dt.float16, tag="warm")
            wps = psa_pool.tile([128, 4, W], mybir.dt.float32, tag="psa")
            for _ in range(20):
                nc.tensor.matmul(
                    wps[:, :, :],
                    warm[:, :128],
                    warm[:, :448],
                    start=True,
                    stop=True,
                    skip_group_check=True,
                )
            nc.vector.tensor_copy(warm[:, :W], wps[:, 0])

            sizes = [2, 2] + [CG] * ((C - 4) // CG)
            assert sum(sizes) == C
            chunks = []
            c0 = 0
            for sz in sizes:
                chunks.append((c0, sz))
                c0 += sz
            for c0, csz in chunks:
                x = xin_pool.tile([128, CG, NS0, W], mybir.dt.float16, tag="x")
                bm = b_pool.tile([128, CG, KW, POB], mybir.dt.float16, tag="bm")
                nc.sync.dma_start(out=bm[:, :csz], in_=bm_d[:, c0 : c0 + csz])
                nc.sync.dma_start(out=x[0:PB, :csz], in_=xa_d[:, c0 : c0 + csz])
                nc.sync.dma_start(
                    out=x[64 : 64 + PB, :csz, :NS1], in_=xb_d[:, c0 : c0 + csz]
                )
                for oi in range((csz + OG - 1) // OG):
                    og = min(OG, csz - oi * OG)
                    osa = osb_pool.tile([128, OG, 4, W], mybir.dt.float16, tag="osa")
                    osc = osb_pool.tile([128, OG, 3, W], mybir.dt.float16, tag="osc")
                    for ci in range(og):
                        cc = oi * OG + ci
                        psa = psa_pool.tile([128, 4, W], mybir.dt.float32, tag="psa")
                        psb = psb_pool.tile([128, 3, W], mybir.dt.float32, tag="psb")
                        for i, kw in enumerate(_KW_ORDER):
                            wi, wj, wo, wp = _w_ranges(kw)
                            for q in range(4):
                                nc.tensor.matmul(
                                    psa[32 * q : 32 * q + 32, :, wo:wp],
                                    bm[0:PB, cc, kw, :],
                                    x[0:PB, cc, 4 * q : 4 * q + 4, wi:wj],
                                    start=(i == 0),
                                    stop=(i == KW - 1),
                                    skip_group_check=(i != 0),
                                    tile_position=(0, 32 * q),
                                )
                            for q in range(4):
                                nc.tensor.matmul(
                                    psb[32 * q : 32 * q + 32, :, wo:wp],
                                    bm[64 : 64 + PB, cc, kw, :],
                                    x[64 : 64 + PB, cc, 3 * q : 3 * q + 3, wi:wj],
                                    start=(i == 0),
                                    stop=(i == KW - 1),
                                    skip_group_check=(i != 0),
                                    tile_position=(64, 32 * q),
                                )
                        nc.vector.tensor_copy(osa[:, ci], psa[:, :, :])
                        nc.scalar.copy(out=osc[:, ci], in_=psb[:, :, :])
                    yc0 = c0 + oi * OG
                    nc.scalar.dma_start(out=ya_d[:, yc0 : yc0 + og], in_=osa[:, :og])
                    nc.scalar.dma_start(out=yb_d[:, yc0 : yc0 + og], in_=osc[:, :og])

    orig_to_json = nc.to_json_bytes
    nc.to_json_bytes = types.MethodType(lambda self: _legalize_bir(orig_to_json()), nc)
    return nc


def _band60(wt, kw):
    """[60, C, 32] band: B[(d_i,h_i), c, (d_o,h_o)] = wt[d_i-d_o, h_i-h_o, kw]
    for the (6,10)-in -> (4,8)-out block, pi = d_i*10 + h_i, po = d_o*8 + h_o."""
    out = np.zeros((PB, C, POB), np.float32)
    do = np.arange(DBO)
    ho = np.arange(HBO)
    po = (do[:, None] * HBO + ho[None, :]).ravel()
    for kd in range(KD):
        for kh in range(KH):
            pi = ((do[:, None] + kd) * HBI + ho[None, :] + kh).ravel()
            out[pi, :, po] = wt[kd, kh, kw, :]
    return out


def _host_prep(x: np.ndarray, w: np.ndarray):
    # x: (4, 16, 112, 112, 64) f32; w: (3, 3, 3, 1, 64) f32
    wt = w[:, :, :, 0, :].astype(np.float32)  # (kd, kh, kw, c)
    b60 = np.stack([_band60(wt, kw) for kw in range(KW)], axis=2)  # [60, C, KW, 32]
    bm = np.zeros((128, C, KW, POB), np.float32)
    bm[0:PB] = b60
    bm[64 : 64 + PB] = b60
    bm = bm.astype(F16)

    xt = np.transpose(x, (0, 4, 1, 2, 3))  # (b, c, d, h, w)

    in_maps = []
    for k in range(N_CORES):
        b = k // 2
        h0 = (k % 2) * HH
        # padded input volume: d 18 (1 zero frame each side), h 58
        xp = np.zeros((C, D_FULL + 2, HH + 2, W), np.float32)
        hlo, hhi = h0 - 1, h0 + HH + 1
        chlo, chhi = max(hlo, 0), min(hhi, H)
        xp[:, 1 : D_FULL + 1, chlo - hlo : chlo - hlo + (chhi - chlo), :] = xt[
            b, :, :, chlo:chhi, :
        ]

        # block (bd, ch): in padded d {4bd..4bd+5}, padded h {8ch..8ch+9}
        xa = np.empty((PB, C, NS0, W), np.float32)
        xb = np.empty((PB, C, NS1, W), np.float32)
        for ch in range(NHB):
            for bd in range(NDB):
                s = xp[:, 4 * bd : 4 * bd + DBI, 8 * ch : 8 * ch + HBI, :]
                v = s.transpose(1, 2, 0, 3).reshape(PB, C, W)
                idx = ch * 4 + bd
                if idx < NS0:
                    xa[:, :, idx, :] = v
                else:
                    xb[:, :, idx - NS0, :] = v
        in_maps.append({"xa": xa.astype(F16), "xb": xb.astype(F16), "bm": bm})
    return in_maps


def _assemble(results):
    y = np.empty((B_FULL, D_FULL, H, W, C), np.float32)
    for k in range(N_CORES):
        b = k // 2
        h0 = (k % 2) * HH
        ya = results[k]["ya"].astype(np.float32)  # [128, C, 4, W]
        yb = results[k]["yb"].astype(np.float32)  # [128, C, 3, W]
        for idx in range(NS0 + NS1):
            ch, bd = divmod(idx, 4)
            if idx < NS0:
                q, s = divmod(idx, 4)
                blk = ya[32 * q : 32 * q + 32, :, s, :]
            else:
                q, s = divmod(idx - NS0, 3)
                blk = yb[32 * q : 32 * q + 32, :, s, :]
            # blk: [32 = (do, ho), C, W]
            v = blk.reshape(DBO, HBO, C, W).transpose(0, 1, 3, 2)
            y[b, 4 * bd : 4 * bd + DBO, h0 + 8 * ch : h0 + 8 * ch + HBO] = v
    return y


def _run(x: np.ndarray, w: np.ndarray, trace: bool = False):
    from concourse.bass_utils import run_bass_kernel_spmd

    in_maps = _host_prep(np.asarray(x), np.asarray(w))
    last_err = None
    for attempt in range(3):
        nc = _build_nc()
        try:
            res = run_bass_kernel_spmd(nc, in_maps, list(range(N_CORES)), trace=trace)
            return _assemble(res.results), res.exec_time_ns
        except Exception as e:  # wedged device is transient; retry fresh
            last_err = e
            print(f"kernel run attempt {attempt} failed: {e!r}", file=sys.stderr)
    raise last_err


def kernel(x: np.ndarray, w: np.ndarray) -> np.ndarray:
    y, _ = _run(x, w, trace=False)
    return y


# revision 15
# speedup vs baseline: 1.0235x; 1.0235x over previous
"""Depthwise 3D conv (3x3x3, SAME, C=64) on 8 Trainium2 NeuronCores.

Strategy
--------
Data-parallel over (batch, h-half): core k handles b = k//2 and output
rows h in [56*(k%2), 56*(k%2)+56), all 16 d frames.

TensorE mapping (8-way tile-packed): the 128x128 PE array runs in 64x32
tile mode - 2 row-groups x 4 column-groups = 8 concurrent tiles. Each
tile computes (4 d, 8 h) output blocks (32 output partitions) from
(6 d, 10 h) input blocks (60 partitions, pi = d_loc*10 + h_loc), with a
60x32 banded stationary B[pi, po] = w[kd, kh, kw, c] applying 9 taps
per streamed column; the 3 kw taps are w-shifts on the moving access
pattern, PSUM-accumulated. Per (channel, kw) the 28 blocks (4 d-blocks
x 7 h-blocks) are spread as 4 blocks/tile on row-group 0 and 3
blocks/tile on row-group 1, so one matmul streams 448 (or 336)
columns. Concurrent tiles give ~1.8x the per-column throughput of the
full-array band scheme, and PSUM quadrants are fully dense (all 128
psum partitions carry real outputs), keeping evacuation cheap.

x is host-gathered (fp16) with full halo duplication in HBM, band
matrices built on host (fp16), device output fp16, host casts back to
fp32.
"""

import json
import sys
import types

if "/opt/trn_rl_repo" not in sys.path:
    sys.path.insert(0, "/opt/trn_rl_repo")

import numpy as np

KD = KH = KW = 3
C = 64
B_FULL, D_FULL, H, W = 4, 16, 112, 112
N_CORES = 8
HH = 56  # output h rows per core
DBO, DBI = 4, 6  # d per block: out frames / in frames
HBO, HBI = 8, 10  # h per block: out rows / in rows
NDB, NHB = 4, 7  # blocks: d 16/4, h 56/8
PB = DBI * HBI  # 60 in-partitions per block
POB = DBO * HBO  # 32 out-partitions per block
NS0, NS1 = 16, 12  # slots on row-group 0 (c_h 0..3) / row-group 1 (c_h 4..6)
CG = 4  # channels per input DMA chunk
OG = 2  # channels per output DMA chunk
F16 = np.float16

_KW_ORDER = [1, 0, 2]  # full-width tap first so PSUM start=True covers all cols


def _legalize_bir(raw: bytes) -> bytes:
    """walrus in this image caps sem waits at 1 per instruction; hoist extra
    waits onto preceding same-engine NoOps (sequencers run them in order)."""
    d = json.loads(raw)
    for fn in d["functions"]:
        for blk in fn["blocks"]:
            out = []
            for inst in blk["instructions"]:
                si = inst.get("sync_info")
                waits = (si or {}).get("on_wait") or []
                if len(waits) > 1:
                    for j, wt in enumerate(waits[:-1]):
                        out.append(
                            {
                                "debug": inst.get("debug", 0),
                                "engine": inst["engine"],
                                "ins": [],
                                "outs": [],
                                "name": f"{inst['name']}-w{j}",
                                "opcode": "NoOp",
                                "sync_info": {"on_wait": [wt], "on_update": []},
                            }
                        )
                    si["on_wait"] = [waits[-1]]
                out.append(inst)
            blk["instructions"] = out
    return json.dumps(d).encode()


def _w_ranges(kw):
    # out[w] += wt[kw] * x[w + kw - 1]
    if kw == 1:
        return 0, W, 0, W
    if kw == 0:
        return 0, W - 1, 1, W
    return 1, W, 0, W - 1


def _build_nc():
    import concourse.bass as bass
    import concourse.mybir as mybir
    import concourse.tile as tile

    nc = bass.Bass()
    # row-group 0: 16 slots (c_h 0..3 x b 0..3); row-group 1: 12 slots
    NCH = 17  # chunks: [2, 2] + [4]*15
    NOG = C // OG
    xa_d = nc.declare_dram_parameter(
        "xa", [NCH, PB, CG, NS0, W], mybir.dt.float16, isOutput=False
    )
    xb_d = nc.declare_dram_parameter(
        "xb", [NCH, PB, CG, NS1, W], mybir.dt.float16, isOutput=False
    )
    bm_d = nc.declare_dram_parameter(
        "bm", [NCH, 128, CG, KW, POB], mybir.dt.float16, isOutput=False
    )
    ya_d = nc.declare_dram_parameter(
        "ya", [NOG, 128, OG, 4, W], mybir.dt.float16, isOutput=True
    )
    yb_d = nc.declare_dram_parameter(
        "yb", [NOG, 128, OG, 3, W], mybir.dt.float16, isOutput=True
    )

    with tile.TileContext(nc) as tc:
        with (
            tc.tile_pool(name="xin", bufs=4) as xin_pool,
            tc.tile_pool(name="bmat", bufs=4) as b_pool,
            tc.tile_pool(name="psa", bufs=4, space="PSUM") as psa_pool,
            tc.tile_pool(name="psb", bufs=4, space="PSUM") as psb_pool,
            tc.tile_pool(name="osb", bufs=4) as osb_pool,
        ):
            # warm the PE (HAM un-throttle) during the first DMA wait
            warm = xin_pool.tile([128, 448], mybir.dt.float16, tag="warm")
            wps = psa_pool.tile([128, 4, W], mybir.dt.float32, tag="psa")
            for _ in range(20):
                nc.tensor.matmul(
                    wps[:, :, :],
                    warm[:, :128],
                    warm[:, :448],
                    start=True,
                    stop=True,
                    skip_group_check=True,
                )
            nc.vector.tensor_copy(warm[:, :W], wps[:, 0])

            sizes = [2, 2] + [CG] * ((C - 4) // CG)
            assert sum(sizes) == C
            chunks = []
            c0 = 0
            for sz in sizes:
                chunks.append((c0, sz))
                c0 += sz
            for ki, (c0, csz) in enumerate(chunks):
                x = xin_pool.tile([128, CG, NS0, W], mybir.dt.float16, tag="x")
                bm = b_pool.tile([128, CG, KW, POB], mybir.dt.float16, tag="bm")
                nc.sync.dma_start(out=bm[:, :csz], in_=bm_d[ki, :, :csz])
                nc.sync.dma_start(out=x[0:PB, :csz], in_=xa_d[ki, :, :csz])
                nc.sync.dma_start(
                    out=x[64 : 64 + PB, :csz, :NS1], in_=xb_d[ki, :, :csz]
                )
                for oi in range((csz + OG - 1) // OG):
                    og = min(OG, csz - oi * OG)
                    osa = osb_pool.tile([128, OG, 4, W], mybir.dt.float16, tag="osa")
                    osc = osb_pool.tile([128, OG, 3, W], mybir.dt.float16, tag="osc")
                    for ci in range(og):
                        cc = oi * OG + ci
                        psa = psa_pool.tile([128, 4, W], mybir.dt.float32, tag="psa")
                        psb = psb_pool.tile([128, 3, W], mybir.dt.float32, tag="psb")
                        for i, kw in enumerate(_KW_ORDER):
                            wi, wj, wo, wp = _w_ranges(kw)
                            for q in range(4):
                                nc.tensor.matmul(
                                    psa[32 * q : 32 * q + 32, :, wo:wp],
                                    bm[0:PB, cc, kw, :],
                                    x[0:PB, cc, 4 * q : 4 * q + 4, wi:wj],
                                    start=(i == 0),
                                    stop=(i == KW - 1),
                                    skip_group_check=(i != 0),
                                    tile_position=(0, 32 * q),
                                )
                            for q in range(4):
                                nc.tensor.matmul(
                                    psb[32 * q : 32 * q + 32, :, wo:wp],
                                    bm[64 : 64 + PB, cc, kw, :],
                                    x[64 : 64 + PB, cc, 3 * q : 3 * q + 3, wi:wj],
                                    start=(i == 0),
                                    stop=(i == KW - 1),
                                    skip_group_check=(i != 0),
                                    tile_position=(64, 32 * q),
                                )
                        nc.vector.tensor_copy(osa[:, ci], psa[:, :, :])
                        nc.scalar.copy(out=osc[:, ci], in_=psb[:, :, :])
                    gi = (c0 + oi * OG) // OG
                    nc.scalar.dma_start(out=ya_d[gi, :, :og], in_=osa[:, :og])
                    nc.scalar.dma_start(out=yb_d[gi, :, :og], in_=osc[:, :og])

    orig_to_json = nc.to_json_bytes
    nc.to_json_bytes = types.MethodType(lambda self: _legalize_bir(orig_to_json()), nc)
    return nc


def _band60(wt, kw):
    """[60, C, 32] band: B[(d_i,h_i), c, (d_o,h_o)] = wt[d_i-d_o, h_i-h_o, kw]
    for the (6,10)-in -> (4,8)-out block, pi = d_i*10 + h_i, po = d_o*8 + h_o."""
    out = np.zeros((PB, C, POB), np.float32)
    do = np.arange(DBO)
    ho = np.arange(HBO)
    po = (do[:, None] * HBO + ho[None, :]).ravel()
    for kd in range(KD):
        for kh in range(KH):
            pi = ((do[:, None] + kd) * HBI + ho[None, :] + kh).ravel()
            out[pi, :, po] = wt[kd, kh, kw, :]
    return out


def _host_prep(x: np.ndarray, w: np.ndarray):
    # x: (4, 16, 112, 112, 64) f32; w: (3, 3, 3, 1, 64) f32
    wt = w[:, :, :, 0, :].astype(np.float32)  # (kd, kh, kw, c)
    b60 = np.stack([_band60(wt, kw) for kw in range(KW)], axis=2)  # [60, C, KW, 32]
    bm = np.zeros((128, C, KW, POB), np.float32)
    bm[0:PB] = b60
    bm[64 : 64 + PB] = b60
    bm = bm.astype(F16)

    xt = np.transpose(x, (0, 4, 1, 2, 3))  # (b, c, d, h, w)

    in_maps = []
    for k in range(N_CORES):
        b = k // 2
        h0 = (k % 2) * HH
        # padded input volume: d 18 (1 zero frame each side), h 58
        xp = np.zeros((C, D_FULL + 2, HH + 2, W), np.float32)
        hlo, hhi = h0 - 1, h0 + HH + 1
        chlo, chhi = max(hlo, 0), min(hhi, H)
        xp[:, 1 : D_FULL + 1, chlo - hlo : chlo - hlo + (chhi - chlo), :] = xt[
            b, :, :, chlo:chhi, :
        ]

        # block (bd, ch): in padded d {4bd..4bd+5}, padded h {8ch..8ch+9}
        xa = np.empty((PB, C, NS0, W), np.float32)
        xb = np.empty((PB, C, NS1, W), np.float32)
        for ch in range(NHB):
            for bd in range(NDB):
                s = xp[:, 4 * bd : 4 * bd + DBI, 8 * ch : 8 * ch + HBI, :]
                v = s.transpose(1, 2, 0, 3).reshape(PB, C, W)
                idx = ch * 4 + bd
                if idx < NS0:
                    xa[:, :, idx, :] = v
                else:
                    xb[:, :, idx - NS0, :] = v
        # chunk-major: [NCH, PB, CG, slots, W]; ragged chunks zero-padded
        sizes = [2, 2] + [CG] * ((C - 4) // CG)
        nch = len(sizes)
        xac = np.zeros((nch, PB, CG, NS0, W), F16)
        xbc = np.zeros((nch, PB, CG, NS1, W), F16)
        bmc = np.zeros((nch, 128, CG, KW, POB), F16)
        cc0 = 0
        for ki, sz in enumerate(sizes):
            xac[ki, :, :sz] = xa[:, cc0 : cc0 + sz].astype(F16)
            xbc[ki, :, :sz] = xb[:, cc0 : cc0 + sz].astype(F16)
            bmc[ki, :, :sz] = bm[:, cc0 : cc0 + sz]
            cc0 += sz
        in_maps.append({"xa": xac, "xb": xbc, "bm": bmc})
    return in_maps


def _assemble(results):
    y = np.empty((B_FULL, D_FULL, H, W, C), np.float32)
    for k in range(N_CORES):
        b = k // 2
        h0 = (k % 2) * HH
        ya = results[k]["ya"].astype(np.float32)  # [NOG, 128, OG, 4, W]
        yb = results[k]["yb"].astype(np.float32)  # [NOG, 128, OG, 3, W]
        ya = ya.transpose(1, 0, 2, 3, 4).reshape(128, C, 4, W)
        yb = yb.transpose(1, 0, 2, 3, 4).reshape(128, C, 3, W)
        for idx in range(NS0 + NS1):
            ch, bd = divmod(idx, 4)
            if idx < NS0:
                q, s = divmod(idx, 4)
                blk = ya[32 * q : 32 * q + 32, :, s, :]
            else:
                q, s = divmod(idx - NS0, 3)
                blk = yb[32 * q : 32 * q + 32, :, s, :]
            # blk: [32 = (do, ho), C, W]
            v = blk.reshape(DBO, HBO, C, W).transpose(0, 1, 3, 2)
            y[b, 4 * bd : 4 * bd + DBO, h0 + 8 * ch : h0 + 8 * ch + HBO] = v
    return y


def _run(x: np.ndarray, w: np.ndarray, trace: bool = False):
    from concourse.bass_utils import run_bass_kernel_spmd

    in_maps = _host_prep(np.asarray(x), np.asarray(w))
    last_err = None
    for attempt in range(3):
        nc = _build_nc()
        try:
            res = run_bass_kernel_spmd(nc, in_maps, list(range(N_CORES)), trace=trace)
            return _assemble(res.results), res.exec_time_ns
        except Exception as e:  # wedged device is transient; retry fresh
            last_err = e
            print(f"kernel run attempt {attempt} failed: {e!r}", file=sys.stderr)
    raise last_err


def kernel(x: np.ndarray, w: np.ndarray) -> np.ndarray:
    y, _ = _run(x, w, trace=False)
    return y
